# revision 1
# baseline (speedup 1.0000x reference)
"""FAGCN (FAConv x3) Trainium2 kernel, 8-core SPMD.

Sharding: nodes partitioned across 8 cores (6250 each, padded to 6272).
Edges assigned to the owner of dst. Per layer: each core computes its
slice of hs = h * dinv and al = h @ att_l, AllGathers the node table,
then runs an edge pass: dma_gather of hs/al rows by src (+ar by dst),
coef = tanh(al_src + ar_dst) * mask, and a one-hot matmul
segment-sum on the TensorEngine (PSUM accumulate per 128-node dst
window).  h_new = dinv * segsum + EPS * raw.

Edge order on each core: [src-half0 | src-half1] (dma_gather indices are
int16, so the 50176-row table is addressed in two halves), within a half
grouped by 128-node dst window, each (half,window) group padded to a
multiple of 128 edges, uniformly across cores (SPMD: one program).
"""
import numpy as np

import concourse.bacc as bacc
import concourse.bass as bass
import concourse.mybir as mybir
import concourse.tile as tile
from concourse.bass_utils import run_bass_kernel_spmd
from concourse.masks import make_identity

F32 = mybir.dt.float32
BF16 = mybir.dt.bfloat16
I16 = mybir.dt.int16

EPS = 0.1


class Cfg:
    def __init__(self, n_nodes, n_edges, in_dim, out_dim, n_layers,
                 n_cores=8, split=32768, csup=8, bf16=False):
        self.BF16 = bf16
        self.N = n_nodes
        self.E = n_edges
        self.IN = in_dim
        self.H = 128
        self.OUT = out_dim
        self.NL = n_layers
        self.NC = n_cores
        self.NV = n_nodes // n_cores          # owned nodes per core
        assert self.NV * n_cores == n_nodes
        self.W = (self.NV + 127) // 128       # dst windows per core
        self.NP = self.W * 128                # padded nodes per core
        self.NPG = self.NP * n_cores          # padded global nodes
        self.KT = in_dim // 128               # k-tiles of the input matmul
        assert in_dim % 128 == 0
        self.SPLIT = split                    # int16 table-half boundary
        self.CSUP = csup                      # chunks per gather call


FULL = Cfg(50000, 600000, 512, 64, 3, bf16=True)


# ----------------------------------------------------------------- planner

def plan_edges(cfg, edge_index):
    """Host-side edge sharding. Returns the uniform chunk schedule and the
    per-core packed arrays."""
    src = edge_index[0].astype(np.int64)
    dst = edge_index[1].astype(np.int64)
    owner = dst // cfg.NV
    remap = (src // cfg.NV) * cfg.NP + (src % cfg.NV)   # padded global row id

    per_core = []
    counts = np.zeros((cfg.NC, 2, cfg.W), np.int64)
    for c in range(cfg.NC):
        m = owner == c
        s_r = remap[m]
        d_l = dst[m] - c * cfg.NV
        w = d_l >> 7
        h = (s_r >= cfg.SPLIT).astype(np.int64)
        order = np.lexsort((d_l, w, h))
        s_r, d_l, w, h = s_r[order], d_l[order], w[order], h[order]
        for hh in range(2):
            for ww in range(cfg.W):
                counts[c, hh, ww] = np.count_nonzero((h == hh) & (w == ww))
        per_core.append((s_r, d_l, h, w))

    nch = np.maximum((counts.max(axis=0) + 127) // 128, 0)  # [2, W] chunks
    nch[counts.max(axis=0) == 0] = 0
    NCH = int(nch.sum())
    EPAD = NCH * 128

    # global chunk meta: (half, window, first_of_group, last_of_group)
    chunk_meta = []
    for hh in range(2):
        for ww in range(cfg.W):
            n = int(nch[hh, ww])
            for k in range(n):
                chunk_meta.append((hh, ww, k == 0, k == n - 1))
    nch0 = int(nch[0].sum())   # chunks in half 0

    cores = []
    for c in range(cfg.NC):
        s_r, d_l, h, w = per_core[c]
        gidx = np.zeros(EPAD, np.int64)     # table row (half-rebased)
        rel = np.full(EPAD, 999.0, np.float32)  # dst rel in window; 999 = pad
        pos = 0
        ptr = 0
        for hh in range(2):
            for ww in range(cfg.W):
                n = counts[c, hh, ww]
                sl = slice(ptr, ptr + n)
                gidx[pos:pos + n] = s_r[sl] - hh * cfg.SPLIT
                rel[pos:pos + n] = (d_l[sl] & 127).astype(np.float32)
                ptr += n
                pos += int(nch[hh, ww]) * 128
        assert ptr == len(s_r)

        def wrap16(v):
            a = v.astype(np.int16).reshape(-1, 16).T.copy()
            return np.tile(a, (8, 1))

        def lanes(v):
            return v.reshape(-1, 128).T.copy()

        cores.append(dict(gidx=wrap16(gidx), rel=lanes(rel)))
    return dict(nch=nch, NCH=NCH, nch0=nch0, EPAD=EPAD,
                chunk_meta=chunk_meta, cores=cores)


def shard_inputs(cfg, inputs, plan):
    """Build per-core in_maps from full inputs."""
    x = np.asarray(inputs["x"], np.float32)
    ei = np.asarray(inputs["edge_index"])
    t1_w = np.asarray(inputs["t1_w"], np.float32)
    t1_b = np.asarray(inputs["t1_b"], np.float32)
    t2_w = np.asarray(inputs["t2_w"], np.float32)
    t2_b = np.asarray(inputs["t2_b"], np.float32)
    att_l = np.asarray(inputs["att_l"], np.float32)
    att_r = np.asarray(inputs["att_r"], np.float32)

    deg_all = np.bincount(ei[1].astype(np.int64), minlength=cfg.N).astype(np.float32)

    w1t = t1_w.T.copy()                      # [IN, H]
    w1t_tiles = w1t.reshape(cfg.KT, 128, cfg.H)
    b1rep = np.broadcast_to(t1_b, (128, cfg.H)).copy()
    alrep = np.stack([np.broadcast_to(att_l[i], (128, cfg.H)) for i in range(cfg.NL)])
    arrep = np.stack([np.broadcast_to(att_r[i], (128, cfg.H)) for i in range(cfg.NL)])
    t2wt = t2_w.T.copy()                     # [H, OUT]
    b2rep = np.broadcast_to(t2_b, (128, cfg.OUT)).copy()
    iota = np.broadcast_to(np.arange(128, dtype=np.float32), (128, 128)).copy()

    in_maps = []
    for c in range(cfg.NC):
        lo = c * cfg.NV
        xc = np.zeros((cfg.NP, cfg.IN), np.float32)
        xc[:cfg.NV] = x[lo:lo + cfg.NV]
        xt = xc.reshape(cfg.W, 128, cfg.KT, 128).transpose(0, 3, 2, 1).copy()
        deg = np.zeros(cfg.NP, np.float32)
        deg[:cfg.NV] = deg_all[lo:lo + cfg.NV]
        pc = plan["cores"][c]
        in_maps.append(dict(
            xt=xt, deg=deg,
            w1t=w1t_tiles, b1rep=b1rep, alrep=alrep, arrep=arrep,
            t2wt=t2wt, b2rep=b2rep, iota=iota,
            gidx=pc["gidx"], rel=pc["rel"],
        ))
    return in_maps


# ----------------------------------------------------------------- builder

def build_program(cfg, plan, skip=frozenset()):
    nch = plan["nch"]
    NCH = plan["NCH"]
    nch0 = plan["nch0"]
    meta = plan["chunk_meta"]
    EPAD = plan["EPAD"]
    W = cfg.W

    nc = bacc.Bacc("TRN2", target_bir_lowering=False, debug=False,
                   num_devices=cfg.NC, num_swdge_queues=2)

    # ---- I/O
    t_xt = nc.dram_tensor("xt", [W, 128, cfg.KT, 128], F32, kind="ExternalInput")
    t_deg = nc.dram_tensor("deg", [cfg.NP], F32, kind="ExternalInput")
    t_w1t = nc.dram_tensor("w1t", [cfg.KT, 128, cfg.H], F32, kind="ExternalInput")
    t_b1 = nc.dram_tensor("b1rep", [128, cfg.H], F32, kind="ExternalInput")
    t_al = nc.dram_tensor("alrep", [cfg.NL, 128, cfg.H], F32, kind="ExternalInput")
    t_ar = nc.dram_tensor("arrep", [cfg.NL, 128, cfg.H], F32, kind="ExternalInput")
    t_t2 = nc.dram_tensor("t2wt", [cfg.H, cfg.OUT], F32, kind="ExternalInput")
    t_b2 = nc.dram_tensor("b2rep", [128, cfg.OUT], F32, kind="ExternalInput")
    t_iota = nc.dram_tensor("iota", [128, 128], F32, kind="ExternalInput")
    t_gidx = nc.dram_tensor("gidx", [128, EPAD // 16], I16, kind="ExternalInput")
    t_rel = nc.dram_tensor("rel", [128, NCH], F32, kind="ExternalInput")
    t_lsm = nc.dram_tensor("lsm", [cfg.NP, cfg.OUT], F32, kind="ExternalOutput")
    t_emb = nc.dram_tensor("emb", [cfg.NP, cfg.OUT], F32, kind="ExternalOutput")

    # ---- internal DRAM
    TDT = BF16 if cfg.BF16 else F32
    RWE = 256 if cfg.BF16 else 192          # table row elems (512B / 768B)
    d_tab_loc = nc.dram_tensor("tab_loc", [cfg.NP, RWE], TDT)
    d_tab_full = nc.dram_tensor("tab_full", [cfg.NPG, RWE], TDT, addr_space="Shared")
    d_ar_loc = nc.dram_tensor("ar_loc", [cfg.NP], F32)

    CS = cfg.CSUP
    rg = [list(range(cfg.NC))]

    with tile.TileContext(nc) as tc:
        with (
            tc.tile_pool(name="const", bufs=1) as cp,
            tc.tile_pool(name="stage", bufs=4) as sp,
            tc.tile_pool(name="gath", bufs=3) as gp,
            tc.tile_pool(name="oh", bufs=8) as op,
            tc.tile_pool(name="small", bufs=4) as mp,
            tc.tile_pool(name="psum", bufs=6, space="PSUM") as pp,
        ):
            # ---------- constants / persistent state
            w1 = cp.tile([128, cfg.KT, cfg.H], F32, tag="w1")
            nc.sync.dma_start(out=w1[:], in_=t_w1t[:].rearrange("k p h -> p k h"))
            b1 = cp.tile([128, cfg.H], F32, tag="b1")
            nc.sync.dma_start(out=b1[:], in_=t_b1[:])
            alr = cp.tile([128, cfg.NL, cfg.H], F32, tag="alr")
            nc.sync.dma_start(out=alr[:], in_=t_al[:].rearrange("l p h -> p l h"))
            arr = cp.tile([128, cfg.NL, cfg.H], F32, tag="arr")
            nc.sync.dma_start(out=arr[:], in_=t_ar[:].rearrange("l p h -> p l h"))
            t2w = cp.tile([cfg.H, cfg.OUT], F32, tag="t2w")
            nc.sync.dma_start(out=t2w[:], in_=t_t2[:])
            b2 = cp.tile([128, cfg.OUT], F32, tag="b2")
            nc.sync.dma_start(out=b2[:], in_=t_b2[:])
            iota = cp.tile([128, 128], F32, tag="iota")
            nc.sync.dma_start(out=iota[:], in_=t_iota[:])
            gidx = cp.tile([128, EPAD // 16], I16, tag="gidx")
            nc.sync.dma_start(out=gidx[:], in_=t_gidx[:])
            ones1 = cp.tile([1, 128], F32, tag="ones1")
            nc.vector.memset(ones1[:], 1.0)
            rel = cp.tile([128, NCH], F32, tag="rel")
            nc.sync.dma_start(out=rel[:], in_=t_rel[:])
            ident = cp.tile([128, 128], F32, tag="ident")
            make_identity(nc, ident[:])

            h_sb = cp.tile([128, W, cfg.H], F32, tag="h")
            raw_sb = cp.tile([128, W, cfg.H], F32, tag="raw")
            acc_sb = cp.tile([128, W, cfg.H], F32, tag="acc")
            dinv = cp.tile([128, W], F32, tag="dinv")
            alc = cp.tile([128, W], F32, tag="alc")
            arc = cp.tile([128, W], F32, tag="arc")

            # ---------- dinv = (deg>0) / sqrt(max(deg,1))
            degt = mp.tile([128, W], F32, tag="degt")
            with nc.allow_non_contiguous_dma(reason="node-col load"):
                nc.sync.dma_start(out=degt[:], in_=t_deg[:].rearrange("(t p) -> p t", p=128))
            dmax = mp.tile([128, W], F32, tag="dmax")
            nc.vector.tensor_scalar_max(dmax[:], degt[:], 1.0)
            nc.scalar.sqrt(dmax[:], dmax[:])
            nc.vector.reciprocal(dmax[:], dmax[:])
            dnz = mp.tile([128, W], F32, tag="dnz")
            nc.vector.tensor_scalar(dnz[:], degt[:], 0.0, None,
                                    op0=mybir.AluOpType.is_gt)
            nc.vector.tensor_tensor(out=dinv[:], in0=dmax[:], in1=dnz[:],
                                    op=mybir.AluOpType.mult)

            # ---------- phase A: h = relu(x @ t1_w.T + b1)
            AB = 7   # node-tiles per x load
            for t0 in range(0, W if "phasea" not in skip else 0, AB):
                nb = min(AB, W - t0)
                xa = gp.tile([128, AB * cfg.KT * 128], F32, tag="xa")
                nc.sync.dma_start(
                    out=xa[:, :nb * cfg.KT * 128],
                    in_=t_xt[t0:t0 + nb].rearrange("w p k n -> p w k n"))
                for ti in range(nb):
                    t = t0 + ti
                    ps = pp.tile([128, cfg.H], F32, tag="ps")
                    for k in range(cfg.KT):
                        o = (ti * cfg.KT + k) * 128
                        nc.tensor.matmul(ps[:], lhsT=xa[:, o:o + 128],
                                         rhs=w1[:, k, :],
                                         start=(k == 0), stop=(k == cfg.KT - 1))
                    hb = sp.tile([128, cfg.H], F32, tag="hb")
                    nc.vector.tensor_add(hb[:], ps[:], b1[:])
                    nc.scalar.activation(h_sb[:, t, :], hb[:],
                                         mybir.ActivationFunctionType.Relu)
                    nc.scalar.mul(raw_sb[:, t, :], h_sb[:, t, :], EPS)

            # ---------- layers
            for li in range(cfg.NL):
                # node-side: al, ar, hs -> tables
                for t in range(W if "nprep" not in skip else 0):
                    tmp = sp.tile([128, cfg.H], F32, tag="nprep")
                    nc.vector.scalar_tensor_tensor(
                        tmp[:], h_sb[:, t, :], 1.0, alr[:, li, :],
                        op0=mybir.AluOpType.mult, op1=mybir.AluOpType.mult,
                        accum_out=alc[:, t:t + 1])
                    nc.vector.scalar_tensor_tensor(
                        tmp[:], h_sb[:, t, :], 1.0, arr[:, li, :],
                        op0=mybir.AluOpType.mult, op1=mybir.AluOpType.mult,
                        accum_out=arc[:, t:t + 1])
                    hst = sp.tile([128, cfg.H], TDT, tag="hst")
                    nc.vector.tensor_scalar_mul(hst[:], h_sb[:, t, :], dinv[:, t:t + 1])
                    nc.sync.dma_start(
                        out=d_tab_loc[t * 128:(t + 1) * 128, :cfg.H], in_=hst[:])
                alx = sp.tile([128, W], TDT, tag="alx")
                nc.vector.tensor_copy(alx[:], alc[:])
                with nc.allow_non_contiguous_dma(reason="node-col store"):
                    nc.sync.dma_start(
                        out=d_tab_loc[:, cfg.H:cfg.H + 1].rearrange(
                            "(t p) c -> p (t c)", p=128),
                        in_=alx[:])
                with nc.allow_non_contiguous_dma(reason="ar-col store"):
                    nc.sync.dma_start(out=d_ar_loc[:].rearrange("(t p) -> p t", p=128),
                                      in_=arc[:])
                arct = cp.tile([1, cfg.NP], F32, tag="arct")
                nc.sync.dma_start(out=arct[:], in_=d_ar_loc[None, :])
                # collective
                if "ag" not in skip:
                    nc.gpsimd.collective_compute(
                        "AllGather", mybir.AluOpType.bypass, replica_groups=rg,
                        ins=[d_tab_loc[:]], outs=[d_tab_full[:]])

                # edge pass
                nc.vector.memset(acc_sb[:], 0.0)
                psw = None
                psar = None
                cur_group = None
                c0 = 0
                while c0 < NCH:
                    nch_call = min(CS, NCH - c0)
                    if meta[c0][0] == 0 and meta[c0 + nch_call - 1][0] == 1:
                        nch_call = nch0 - c0          # don't span the half split
                    half = meta[c0][0]
                    ne = nch_call * 128
                    ghs = gp.tile([128, CS * RWE], TDT, tag="ghs")
                    tab_src = d_tab_full[:] if half == 0 else d_tab_full[cfg.SPLIT:, :]
                    i0, i1 = c0 * 8, (c0 + nch_call) * 8
                    if "gather" in skip:
                        nc.vector.memset(ghs[:], 0.0)
                    if "gather" not in skip:
                     nc.gpsimd.dma_gather(
                        out_ap=ghs[:, :nch_call * RWE].rearrange(
                            "p (c e) -> p c e", e=RWE),
                        in_ap=tab_src, idxs_ap=gidx[:, i0:i1],
                        num_idxs=ne, num_idxs_reg=ne, elem_size=RWE,
                        queue_num=(c0 // CS) % 2)
                    for j in range(nch_call if "chunk" not in skip else 0):
                        ci = c0 + j
                        hh, ww, first, last = meta[ci]
                        if (hh, ww) != cur_group:
                            # ar_rep[p, n] = ar[window w, node n] via rank-1 matmul
                            psar = pp.tile([128, 128], F32, tag="ps")
                            nc.tensor.matmul(
                                psar[:], lhsT=ones1[:],
                                rhs=arct[0:1, ww * 128:(ww + 1) * 128],
                                start=True, stop=True)
                            cur_group = (hh, ww)
                        tt = op.tile([128, 128], F32, tag="tt")
                        nc.scalar.activation(
                            tt[:], psar[:], mybir.ActivationFunctionType.Tanh,
                            bias=ghs[:, j * RWE + cfg.H:j * RWE + cfg.H + 1])
                        ohp = op.tile([128, 128], TDT, tag="ohp")
                        nc.vector.scalar_tensor_tensor(
                            ohp[:], iota[:], rel[:, ci:ci + 1], tt[:],
                            op0=mybir.AluOpType.is_equal,
                            op1=mybir.AluOpType.mult)
                        if first:
                            psw = pp.tile([128, cfg.H], F32, tag="ps")
                        nc.tensor.matmul(psw[:], lhsT=ohp[:],
                                         rhs=ghs[:, j * RWE:j * RWE + cfg.H],
                                         start=first, stop=last)
                        if last:
                            nc.vector.tensor_add(acc_sb[:, ww, :],
                                                 acc_sb[:, ww, :], psw[:])
                    c0 += nch_call
                # h_new = dinv * acc + raw_eps   (raw_eps = EPS*h0, precomputed)
                for t in range(W if "nprep" not in skip else 0):
                    nc.vector.scalar_tensor_tensor(
                        h_sb[:, t, :], acc_sb[:, t, :], dinv[:, t:t + 1],
                        raw_sb[:, t, :],
                        op0=mybir.AluOpType.mult, op1=mybir.AluOpType.add)

            # ---------- phase C: emb = h @ t2_w.T + b2; lsm = log_softmax
            for t in range(W if "phasec" not in skip else 0):
                pst = pp.tile([128, 128], F32, tag="ps")
                nc.tensor.transpose(out=pst[:], in_=h_sb[:, t, :], identity=ident[:])
                ht = sp.tile([128, 128], F32, tag="ht")
                nc.vector.tensor_copy(ht[:], pst[:])
                pse = pp.tile([128, cfg.OUT], F32, tag="ps")
                nc.tensor.matmul(pse[:], lhsT=ht[:], rhs=t2w[:], start=True, stop=True)
                emb = sp.tile([128, cfg.OUT], F32, tag="embt")
                nc.vector.tensor_add(emb[:], pse[:], b2[:])
                nc.sync.dma_start(out=t_emb[t * 128:(t + 1) * 128, :], in_=emb[:])
                mx = mp.tile([128, 1], F32, tag="mx")
                nc.vector.tensor_reduce(mx[:], emb[:], axis=mybir.AxisListType.X,
                                        op=mybir.AluOpType.max)
                sh = sp.tile([128, cfg.OUT], F32, tag="sh")
                nc.vector.tensor_scalar(sh[:], emb[:], mx[:], None,
                                        op0=mybir.AluOpType.subtract)
                ex = sp.tile([128, cfg.OUT], F32, tag="ex")
                nc.scalar.activation(ex[:], sh[:], mybir.ActivationFunctionType.Exp)
                sm = mp.tile([128, 1], F32, tag="sm")
                nc.vector.tensor_reduce(sm[:], ex[:], axis=mybir.AxisListType.X,
                                        op=mybir.AluOpType.add)
                nc.scalar.activation(sm[:], sm[:], mybir.ActivationFunctionType.Ln)
                nc.vector.tensor_scalar(sh[:], sh[:], sm[:], None,
                                        op0=mybir.AluOpType.subtract)
                nc.sync.dma_start(out=t_lsm[t * 128:(t + 1) * 128, :], in_=sh[:])

    nc.finalize()
    return nc




# ------------------------------------------------------- cached PJRT runner

def _make_runner(nc, n_cores):
    """Like bass2jax.run_bass_via_pjrt, but builds the jitted executable once
    so repeated calls don't re-trace/re-compile."""
    import jax
    import concourse.mybir as mb
    from jax.sharding import Mesh, PartitionSpec
    from jax.experimental.shard_map import shard_map
    from concourse.bass2jax import (install_neuronx_cc_hook, partition_id_tensor,
                                    _bass_exec_p)
    install_neuronx_cc_hook()
    partition_name = nc.partition_id_tensor.name if nc.partition_id_tensor else None
    in_names, out_names, out_avals, zero_outs = [], [], [], []
    for alloc in nc.m.functions[0].allocations:
        if not isinstance(alloc, mb.MemoryLocationSet):
            continue
        name = alloc.memorylocations[0].name
        if alloc.kind == "ExternalInput":
            if name != partition_name:
                in_names.append(name)
        elif alloc.kind == "ExternalOutput":
            out_names.append(name)
            shape = tuple(alloc.tensor_shape)
            dtype = mb.dt.np(alloc.dtype)
            out_avals.append(jax.core.ShapedArray(shape, dtype))
            zero_outs.append(np.zeros(shape, dtype))
    n_params = len(in_names)
    n_outs = len(out_avals)
    all_in_names = list(in_names) + list(out_names)
    if partition_name is not None:
        all_in_names.append(partition_name)
    donate = tuple(range(n_params, n_params + n_outs))

    def _body(*args):
        operands = list(args)
        if partition_name is not None:
            operands.append(partition_id_tensor())
        return tuple(_bass_exec_p.bind(
            *operands, out_avals=tuple(out_avals), in_names=tuple(all_in_names),
            out_names=tuple(out_names), lowering_input_output_aliases=(),
            sim_require_finite=True, sim_require_nnan=True, nc=nc))

    devices = jax.devices()[:n_cores]
    mesh = Mesh(np.asarray(devices), ("core",))
    in_specs = (PartitionSpec("core"),) * (n_params + n_outs)
    out_specs = (PartitionSpec("core"),) * n_outs
    sharded = jax.jit(
        shard_map(_body, mesh=mesh, in_specs=in_specs, out_specs=out_specs,
                  check_rep=False),
        donate_argnums=donate, keep_unused=True)

    def call(in_maps):
        concat_in = [
            np.concatenate([np.asarray(in_maps[c][k]) for c in range(n_cores)], 0)
            for k in in_names
        ]
        concat_zeros = [
            np.zeros((n_cores * z.shape[0], *z.shape[1:]), z.dtype)
            for z in zero_outs
        ]
        out_arrs = sharded(*concat_in, *concat_zeros)
        jax.block_until_ready(out_arrs)
        return [
            {k: np.asarray(out_arrs[i]).reshape(n_cores, *out_avals[i].shape)[c]
             for i, k in enumerate(out_names)}
            for c in range(n_cores)
        ]

    return call


# Measured by layer-amplification differencing (t_attr.py); updated as the
# kernel is optimized.
HW_EXEC_NS_ESTIMATE = 2127000

# ----------------------------------------------------------------- entry

_CACHE = {}


def run(cfg, inputs, trace=False):
    ei = np.asarray(inputs["edge_index"])
    key = (cfg.N, cfg.E, cfg.NL, hash(ei.tobytes()))
    if key in _CACHE:
        runner, plan = _CACHE[key]
    else:
        plan = plan_edges(cfg, ei)
        nc = build_program(cfg, plan)
        runner = _make_runner(nc, cfg.NC)
        _CACHE[key] = (runner, plan)
    in_maps = shard_inputs(cfg, inputs, plan)
    results = runner(in_maps)
    lsm = np.concatenate([results[c]["lsm"][:cfg.NV] for c in range(cfg.NC)], 0)
    emb = np.concatenate([results[c]["emb"][:cfg.NV] for c in range(cfg.NC)], 0)
    return (lsm, emb), None


def kernel(**inputs):
    (lsm, emb), _ = run(FULL, inputs)
    return lsm, emb



# revision 4
# speedup vs baseline: 1.4568x; 1.4568x over previous
"""FAGCN (FAConv x3) Trainium2 kernel, 8-core SPMD.

Sharding: nodes partitioned across 8 cores (6250 each, padded to 6272).
Edges assigned to the owner of dst. Per layer each core computes its
slice of the node table (rows = [h*dinv (128 bf16) | al | pad] = 512B),
AllGathers it, then runs an edge pass: dma_gather of table rows by src,
coef = tanh(al_src + ar_dst) * mask, and a one-hot matmul segment-sum on
the TensorEngine (PSUM accumulate per 128-node dst window).
h_new = dinv * segsum + EPS * raw.

Collective/compute overlap: the per-core table is stored as two
row-disjoint tensors (A: rows 0..3200, B: 3200..6272, so each AllGather
output stays under the int16 gather-index reach with no rebasing).
Edges are ordered [locA | locB | remA | remB] (src on own core / remote,
table half A/B), each grouped by 128-node dst window. AllGather-A is
issued as soon as the first 3200 rows are prepped (mid phase-A / mid
combine-loop), AllGather-B at the end; the local segments gather from
the local tables and run while the collectives are in flight, so the
remote segments' gathers rarely wait.

Gather calls are 8 chunks (1024 rows) each -- the SWDGE descriptor
carveout is 16KB = 1024 descriptors, a hard ucode limit -- striped over
4 swdge queues with a GLOBAL call counter (tile_sem_assignment binds
Pool-DMA instructions to DMASW lanes round-robin mod 8 program-wide, so
queue must be counter % 4 to keep each lane on one queue), and 6 gather
buffers to keep several calls in flight.
"""
import numpy as np

import concourse.bacc as bacc
import concourse.bass as bass
import concourse.mybir as mybir
import concourse.tile as tile
from concourse.bass_utils import run_bass_kernel_spmd
from concourse.masks import make_identity

F32 = mybir.dt.float32
BF16 = mybir.dt.bfloat16
I16 = mybir.dt.int16

EPS = 0.1


class Cfg:
    def __init__(self, n_nodes, n_edges, in_dim, out_dim, n_layers,
                 n_cores=8, wa=25, csup=8, nq=4, gbufs=6):
        self.N = n_nodes
        self.E = n_edges
        self.IN = in_dim
        self.H = 128
        self.OUT = out_dim
        self.NL = n_layers
        self.NC = n_cores
        self.NV = n_nodes // n_cores          # owned nodes per core
        assert self.NV * n_cores == n_nodes
        self.W = (self.NV + 127) // 128       # dst windows per core
        self.NP = self.W * 128                # padded nodes per core
        self.KT = in_dim // 128               # k-tiles of the input matmul
        assert in_dim % 128 == 0
        self.WA = wa                          # windows in table half A
        self.RA = wa * 128
        self.RB = self.NP - self.RA
        assert n_cores * self.RA <= 32768 and n_cores * self.RB <= 32768
        self.CSUP = csup                      # chunks per gather call (<=8)
        self.NQ = nq                          # swdge queues (<=4)
        self.GBUFS = gbufs                    # gather tile buffers


FULL = Cfg(50000, 600000, 512, 64, 3)


# ----------------------------------------------------------------- planner

def plan_edges(cfg, edge_index):
    """Host-side edge sharding: 4 segments x dst-window groups, uniform
    (max-over-cores) chunk schedule so the SPMD program is shared."""
    src = edge_index[0].astype(np.int64)
    dst = edge_index[1].astype(np.int64)
    owner = dst // cfg.NV

    NSEG = 4
    per_core = []
    counts = np.zeros((cfg.NC, NSEG, cfg.W), np.int64)
    for c in range(cfg.NC):
        m = owner == c
        s = src[m]
        d_l = dst[m] - c * cfg.NV
        w = d_l >> 7
        c_src = s // cfg.NV
        r = s % cfg.NV
        seg = np.where(c_src == c,
                       np.where(r < cfg.RA, 0, 1),
                       np.where(r < cfg.RA, 2, 3))
        order = np.lexsort((d_l, w, seg))
        s, d_l, w, seg, c_src, r = (a[order] for a in (s, d_l, w, seg, c_src, r))
        gv = np.select(
            [seg == 0, seg == 1, seg == 2, seg == 3],
            [r, r - cfg.RA, c_src * cfg.RA + r, c_src * cfg.RB + (r - cfg.RA)])
        for sg in range(NSEG):
            for ww in range(cfg.W):
                counts[c, sg, ww] = np.count_nonzero((seg == sg) & (w == ww))
        per_core.append((gv, d_l))

    nch = np.maximum((counts.max(axis=0) + 127) // 128, 0)
    nch[counts.max(axis=0) == 0] = 0
    NCH = int(nch.sum())
    EPAD = NCH * 128

    chunk_meta = []       # (seg, window, first_of_group, last_of_group)
    first_grp = {}        # window -> first segment with chunks
    for sg in range(NSEG):
        for ww in range(cfg.W):
            n = int(nch[sg, ww])
            if n and ww not in first_grp:
                first_grp[ww] = sg
            for k in range(n):
                chunk_meta.append((sg, ww, k == 0, k == n - 1))
    seg_end = np.cumsum(nch.sum(axis=1)).astype(int)

    cores = []
    for c in range(cfg.NC):
        gv, d_l = per_core[c]
        gidx = np.zeros(EPAD, np.int64)
        rel = np.full(EPAD, 999.0, np.float32)  # dst rel in window; 999 = pad
        pos = 0
        ptr = 0
        for sg in range(NSEG):
            for ww in range(cfg.W):
                n = counts[c, sg, ww]
                sl = slice(ptr, ptr + n)
                gidx[pos:pos + n] = gv[sl]
                rel[pos:pos + n] = (d_l[sl] & 127).astype(np.float32)
                ptr += n
                pos += int(nch[sg, ww]) * 128
        assert ptr == len(gv)

        def wrap16(v):
            a = v.astype(np.int16).reshape(-1, 16).T.copy()
            return np.tile(a, (8, 1))

        def lanes(v):
            return v.reshape(-1, 128).T.copy()

        cores.append(dict(gidx=wrap16(gidx), rel=lanes(rel)))
    return dict(nch=nch, NCH=NCH, EPAD=EPAD, seg_end=list(seg_end),
                chunk_meta=chunk_meta, first_grp=first_grp, cores=cores)


def shard_inputs(cfg, inputs, plan):
    """Build per-core in_maps from full inputs."""
    x = np.asarray(inputs["x"], np.float32)
    ei = np.asarray(inputs["edge_index"])
    t1_w = np.asarray(inputs["t1_w"], np.float32)
    t1_b = np.asarray(inputs["t1_b"], np.float32)
    t2_w = np.asarray(inputs["t2_w"], np.float32)
    t2_b = np.asarray(inputs["t2_b"], np.float32)
    att_l = np.asarray(inputs["att_l"], np.float32)
    att_r = np.asarray(inputs["att_r"], np.float32)

    deg_all = np.bincount(ei[1].astype(np.int64), minlength=cfg.N).astype(np.float32)

    w1t = t1_w.T.copy()
    w1t_tiles = w1t.reshape(cfg.KT, 128, cfg.H)
    b1rep = np.broadcast_to(t1_b, (128, cfg.H)).copy()
    alrep = np.stack([np.broadcast_to(att_l[i % att_l.shape[0]], (128, cfg.H))
                      for i in range(cfg.NL)])
    arrep = np.stack([np.broadcast_to(att_r[i % att_r.shape[0]], (128, cfg.H))
                      for i in range(cfg.NL)])
    t2wt = t2_w.T.copy()
    b2rep = np.broadcast_to(t2_b, (128, cfg.OUT)).copy()
    iota = np.broadcast_to(np.arange(128, dtype=np.float32), (128, 128)).copy()

    in_maps = []
    for c in range(cfg.NC):
        lo = c * cfg.NV
        xc = np.zeros((cfg.NP, cfg.IN), np.float32)
        xc[:cfg.NV] = x[lo:lo + cfg.NV]
        xt = xc.reshape(cfg.W, 128, cfg.KT, 128).transpose(0, 3, 2, 1).copy()
        deg = np.zeros(cfg.NP, np.float32)
        deg[:cfg.NV] = deg_all[lo:lo + cfg.NV]
        pc = plan["cores"][c]
        in_maps.append(dict(
            xt=xt, deg=deg,
            w1t=w1t_tiles, b1rep=b1rep, alrep=alrep, arrep=arrep,
            t2wt=t2wt, b2rep=b2rep, iota=iota,
            gidx=pc["gidx"], rel=pc["rel"],
        ))
    return in_maps


# ----------------------------------------------------------------- builder

def build_program(cfg, plan, skip=frozenset()):
    NCH = plan["NCH"]
    meta = plan["chunk_meta"]
    first_grp = plan["first_grp"]
    seg_end = plan["seg_end"]
    EPAD = plan["EPAD"]
    W = cfg.W
    WA = cfg.WA
    WB = W - WA
    RWE = 256  # bf16 row: 512B = [hs(128) | al | pad]

    nc = bacc.Bacc("TRN2", target_bir_lowering=False, debug=False,
                   num_devices=cfg.NC, num_swdge_queues=cfg.NQ)

    t_xt = nc.dram_tensor("xt", [W, 128, cfg.KT, 128], F32, kind="ExternalInput")
    t_deg = nc.dram_tensor("deg", [cfg.NP], F32, kind="ExternalInput")
    t_w1t = nc.dram_tensor("w1t", [cfg.KT, 128, cfg.H], F32, kind="ExternalInput")
    t_b1 = nc.dram_tensor("b1rep", [128, cfg.H], F32, kind="ExternalInput")
    t_al = nc.dram_tensor("alrep", [cfg.NL, 128, cfg.H], F32, kind="ExternalInput")
    t_ar = nc.dram_tensor("arrep", [cfg.NL, 128, cfg.H], F32, kind="ExternalInput")
    t_t2 = nc.dram_tensor("t2wt", [cfg.H, cfg.OUT], F32, kind="ExternalInput")
    t_b2 = nc.dram_tensor("b2rep", [128, cfg.OUT], F32, kind="ExternalInput")
    t_iota = nc.dram_tensor("iota", [128, 128], F32, kind="ExternalInput")
    t_gidx = nc.dram_tensor("gidx", [128, EPAD // 16], I16, kind="ExternalInput")
    t_rel = nc.dram_tensor("rel", [128, NCH], F32, kind="ExternalInput")
    t_lsm = nc.dram_tensor("lsm", [cfg.NP, cfg.OUT], F32, kind="ExternalOutput")
    t_emb = nc.dram_tensor("emb", [cfg.NP, cfg.OUT], F32, kind="ExternalOutput")

    d_locA = nc.dram_tensor("tab_locA", [cfg.RA, RWE], BF16)
    d_locB = nc.dram_tensor("tab_locB", [cfg.RB, RWE], BF16)
    d_fullA = nc.dram_tensor("tab_fullA", [cfg.NC * cfg.RA, RWE], BF16,
                             addr_space="Shared")
    d_fullB = nc.dram_tensor("tab_fullB", [cfg.NC * cfg.RB, RWE], BF16,
                             addr_space="Shared")
    d_ar_loc = nc.dram_tensor("ar_loc", [cfg.NP], F32)

    CS = cfg.CSUP
    rg = [list(range(cfg.NC))]

    with tile.TileContext(nc) as tc:
        with (
            tc.tile_pool(name="const", bufs=1) as cp,
            tc.tile_pool(name="stage", bufs=4) as sp,
            tc.tile_pool(name="xld", bufs=2) as xp,
            tc.tile_pool(name="gath", bufs=cfg.GBUFS) as gp,
            tc.tile_pool(name="oh", bufs=8) as op,
            tc.tile_pool(name="small", bufs=4) as mp,
            tc.tile_pool(name="psum", bufs=6, space="PSUM") as pp,
        ):
            w1 = cp.tile([128, cfg.KT, cfg.H], F32, tag="w1")
            nc.sync.dma_start(out=w1[:], in_=t_w1t[:].rearrange("k p h -> p k h"))
            b1 = cp.tile([128, cfg.H], F32, tag="b1")
            nc.sync.dma_start(out=b1[:], in_=t_b1[:])
            alr = cp.tile([128, cfg.NL, cfg.H], F32, tag="alr")
            nc.sync.dma_start(out=alr[:], in_=t_al[:].rearrange("l p h -> p l h"))
            arr = cp.tile([128, cfg.NL, cfg.H], F32, tag="arr")
            nc.sync.dma_start(out=arr[:], in_=t_ar[:].rearrange("l p h -> p l h"))
            t2w = cp.tile([cfg.H, cfg.OUT], F32, tag="t2w")
            nc.sync.dma_start(out=t2w[:], in_=t_t2[:])
            b2 = cp.tile([128, cfg.OUT], F32, tag="b2")
            nc.sync.dma_start(out=b2[:], in_=t_b2[:])
            iota = cp.tile([128, 128], F32, tag="iota")
            nc.sync.dma_start(out=iota[:], in_=t_iota[:])
            gidx = cp.tile([128, EPAD // 16], I16, tag="gidx")
            nc.sync.dma_start(out=gidx[:], in_=t_gidx[:])
            ones1 = cp.tile([1, 128], F32, tag="ones1")
            nc.vector.memset(ones1[:], 1.0)
            rel = cp.tile([128, NCH], F32, tag="rel")
            nc.sync.dma_start(out=rel[:], in_=t_rel[:])
            ident = cp.tile([128, 128], F32, tag="ident")
            make_identity(nc, ident[:])

            h_sb = cp.tile([128, W, cfg.H], F32, tag="h")
            raw_sb = cp.tile([128, W, cfg.H], F32, tag="raw")
            acc_sb = cp.tile([128, W, cfg.H], F32, tag="acc")
            dinv = cp.tile([128, W], F32, tag="dinv")
            alc = cp.tile([128, W], F32, tag="alc")
            arc = cp.tile([128, W], F32, tag="arc")

            if "gather0" in skip:
                ghs0 = cp.tile([128, CS * RWE], BF16, tag="ghs0")
                nc.vector.memset(ghs0[:], 0.0)

            # ---------- dinv = (deg>0) / sqrt(max(deg,1))
            degt = mp.tile([128, W], F32, tag="degt")
            with nc.allow_non_contiguous_dma(reason="node-col load"):
                nc.sync.dma_start(out=degt[:], in_=t_deg[:].rearrange("(t p) -> p t", p=128))
            dmax = mp.tile([128, W], F32, tag="dmax")
            nc.vector.tensor_scalar_max(dmax[:], degt[:], 1.0)
            nc.scalar.sqrt(dmax[:], dmax[:])
            nc.vector.reciprocal(dmax[:], dmax[:])
            dnz = mp.tile([128, W], F32, tag="dnz")
            nc.vector.tensor_scalar(dnz[:], degt[:], 0.0, None,
                                    op0=mybir.AluOpType.is_gt)
            nc.vector.tensor_tensor(out=dinv[:], in0=dmax[:], in1=dnz[:],
                                    op=mybir.AluOpType.mult)

            def nprep(t, li):
                """al/ar accum + hs row store for window t of layer li."""
                tmp = sp.tile([128, cfg.H], F32, tag="nprep")
                nc.vector.scalar_tensor_tensor(
                    tmp[:], h_sb[:, t, :], 1.0, alr[:, li, :],
                    op0=mybir.AluOpType.mult, op1=mybir.AluOpType.mult,
                    accum_out=alc[:, t:t + 1])
                nc.vector.scalar_tensor_tensor(
                    tmp[:], h_sb[:, t, :], 1.0, arr[:, li, :],
                    op0=mybir.AluOpType.mult, op1=mybir.AluOpType.mult,
                    accum_out=arc[:, t:t + 1])
                hst = sp.tile([128, cfg.H], BF16, tag="hst")
                nc.vector.tensor_scalar_mul(hst[:], h_sb[:, t, :], dinv[:, t:t + 1])
                if t < WA:
                    dst_ap = d_locA[t * 128:(t + 1) * 128, :cfg.H]
                else:
                    tb = t - WA
                    dst_ap = d_locB[tb * 128:(tb + 1) * 128, :cfg.H]
                nc.sync.dma_start(out=dst_ap, in_=hst[:])

            def colstore_a():
                alx = sp.tile([128, WA], BF16, tag="alxA")
                nc.vector.tensor_copy(alx[:], alc[:, :WA])
                with nc.allow_non_contiguous_dma(reason="al-col store A"):
                    nc.sync.dma_start(
                        out=d_locA[:, cfg.H:cfg.H + 1].rearrange(
                            "(t p) c -> p (t c)", p=128),
                        in_=alx[:])

            def colstore_b():
                alx = sp.tile([128, WB], BF16, tag="alxB")
                nc.vector.tensor_copy(alx[:], alc[:, WA:])
                with nc.allow_non_contiguous_dma(reason="al-col store B"):
                    nc.sync.dma_start(
                        out=d_locB[:, cfg.H:cfg.H + 1].rearrange(
                            "(t p) c -> p (t c)", p=128),
                        in_=alx[:])

            def ag_a():
                if "ag" not in skip:
                    nc.gpsimd.collective_compute(
                        "AllGather", mybir.AluOpType.bypass, replica_groups=rg,
                        ins=[d_locA[:]], outs=[d_fullA[:]])

            def ag_b():
                if "ag" not in skip:
                    nc.gpsimd.collective_compute(
                        "AllGather", mybir.AluOpType.bypass, replica_groups=rg,
                        ins=[d_locB[:]], outs=[d_fullB[:]])

            # ---------- phase A: h = relu(x@W1+b1), fused layer-0 nprep + AGs
            AB = 7
            for t0 in range(0, W if "phasea" not in skip else 0, AB):
                nb = min(AB, W - t0)
                xa = xp.tile([128, AB * cfg.KT * 128], F32, tag="xa")
                nc.sync.dma_start(
                    out=xa[:, :nb * cfg.KT * 128],
                    in_=t_xt[t0:t0 + nb].rearrange("w p k n -> p w k n"))
                for ti in range(nb):
                    t = t0 + ti
                    ps = pp.tile([128, cfg.H], F32, tag="ps")
                    for k in range(cfg.KT):
                        o = (ti * cfg.KT + k) * 128
                        nc.tensor.matmul(ps[:], lhsT=xa[:, o:o + 128],
                                         rhs=w1[:, k, :],
                                         start=(k == 0), stop=(k == cfg.KT - 1))
                    hb = sp.tile([128, cfg.H], F32, tag="hb")
                    nc.vector.tensor_add(hb[:], ps[:], b1[:])
                    nc.scalar.activation(h_sb[:, t, :], hb[:],
                                         mybir.ActivationFunctionType.Relu)
                    nc.scalar.mul(raw_sb[:, t, :], h_sb[:, t, :], EPS)
                    if "nprep" not in skip:
                        nprep(t, 0)
                        if t == WA - 1:
                            colstore_a()
                            ag_a()
                        if t == W - 1:
                            colstore_b()
                            ag_b()

            # ---------- layers
            # Global gather-call counter: tile_sem_assignment binds Pool-DMA
            # instructions to DMASW lanes round-robin mod 8 program-wide, so
            # queue must be (global counter) % NQ to keep each lane on a
            # single queue (NQ divides 8).
            gcall = [0]

            def gather_tab(sg):
                return [d_locA[:], d_locB[:], d_fullA[:], d_fullB[:]][sg]

            for li in range(cfg.NL):
                # ar flatten roundtrip: arc [128, W] -> d_ar_loc -> arct [1, NP]
                arct = cp.tile([1, cfg.NP], F32, tag="arct")
                if "nprep" not in skip:
                    with nc.allow_non_contiguous_dma(reason="ar-col store"):
                        nc.sync.dma_start(
                            out=d_ar_loc[:].rearrange("(t p) -> p t", p=128),
                            in_=arc[:])
                    nc.sync.dma_start(out=arct[:], in_=d_ar_loc[None, :])
                elif "chunk" not in skip:
                    nc.vector.memset(arct[:], 0.0)

                # edge pass
                psw = None
                psar = None
                cur_group = None
                c0 = 0
                while c0 < NCH:
                    bound = next(b for b in seg_end if b > c0)
                    nch_call = min(CS, NCH - c0, bound - c0)
                    sg = meta[c0][0]
                    ne = nch_call * 128
                    if "gather0" in skip:
                        ghs = ghs0
                    else:
                        ghs = gp.tile([128, CS * RWE], BF16, tag="ghs")
                        i0, i1 = c0 * 8, (c0 + nch_call) * 8
                        nc.gpsimd.dma_gather(
                            out_ap=ghs[:, :nch_call * RWE].rearrange(
                                "p (c e) -> p c e", e=RWE),
                            in_ap=gather_tab(sg), idxs_ap=gidx[:, i0:i1],
                            num_idxs=ne, num_idxs_reg=ne, elem_size=RWE,
                            queue_num=gcall[0] % cfg.NQ)
                        gcall[0] += 1
                    for j in range(nch_call if "chunk" not in skip else 0):
                        ci = c0 + j
                        sgj, ww, first, last = meta[ci]
                        if (sgj, ww) != cur_group:
                            # ar_rep[p, n] = ar[window, node n] via rank-1 mm
                            psar = pp.tile([128, 128], F32, tag="ps")
                            nc.tensor.matmul(
                                psar[:], lhsT=ones1[:],
                                rhs=arct[0:1, ww * 128:(ww + 1) * 128],
                                start=True, stop=True)
                            cur_group = (sgj, ww)
                        tt = op.tile([128, 128], F32, tag="tt")
                        nc.scalar.activation(
                            tt[:], psar[:], mybir.ActivationFunctionType.Tanh,
                            bias=ghs[:, j * RWE + cfg.H:j * RWE + cfg.H + 1])
                        ohp = op.tile([128, 128], BF16, tag="ohp")
                        nc.vector.scalar_tensor_tensor(
                            ohp[:], iota[:], rel[:, ci:ci + 1], tt[:],
                            op0=mybir.AluOpType.is_equal,
                            op1=mybir.AluOpType.mult)
                        if first:
                            psw = pp.tile([128, cfg.H], F32, tag="ps")
                        nc.tensor.matmul(psw[:], lhsT=ohp[:],
                                         rhs=ghs[:, j * RWE:j * RWE + cfg.H],
                                         start=first, stop=last)
                        if last:
                            if first_grp.get(ww) == sgj:
                                nc.vector.tensor_copy(acc_sb[:, ww, :], psw[:])
                            else:
                                nc.vector.tensor_add(acc_sb[:, ww, :],
                                                     acc_sb[:, ww, :], psw[:])
                    c0 += nch_call

                # combine + next-layer nprep fused; early AGs for layer li+1
                for t in range(W if "nprep" not in skip else 0):
                    if t in first_grp:
                        nc.vector.scalar_tensor_tensor(
                            h_sb[:, t, :], acc_sb[:, t, :], dinv[:, t:t + 1],
                            raw_sb[:, t, :],
                            op0=mybir.AluOpType.mult, op1=mybir.AluOpType.add)
                    else:
                        nc.vector.tensor_copy(h_sb[:, t, :], raw_sb[:, t, :])
                    if li + 1 < cfg.NL:
                        nprep(t, li + 1)
                        if t == WA - 1:
                            colstore_a()
                            ag_a()
                        if t == W - 1:
                            colstore_b()
                            ag_b()

            # ---------- phase C: emb = h @ t2_w.T + b2; lsm = log_softmax
            for t in range(W if "phasec" not in skip else 0):
                pst = pp.tile([128, 128], F32, tag="ps")
                nc.tensor.transpose(out=pst[:], in_=h_sb[:, t, :], identity=ident[:])
                ht = sp.tile([128, 128], F32, tag="ht")
                nc.vector.tensor_copy(ht[:], pst[:])
                pse = pp.tile([128, cfg.OUT], F32, tag="ps")
                nc.tensor.matmul(pse[:], lhsT=ht[:], rhs=t2w[:], start=True, stop=True)
                emb = sp.tile([128, cfg.OUT], F32, tag="embt")
                nc.vector.tensor_add(emb[:], pse[:], b2[:])
                nc.sync.dma_start(out=t_emb[t * 128:(t + 1) * 128, :], in_=emb[:])
                mx = mp.tile([128, 1], F32, tag="mx")
                nc.vector.tensor_reduce(mx[:], emb[:], axis=mybir.AxisListType.X,
                                        op=mybir.AluOpType.max)
                sh = sp.tile([128, cfg.OUT], F32, tag="sh")
                nc.vector.tensor_scalar(sh[:], emb[:], mx[:], None,
                                        op0=mybir.AluOpType.subtract)
                ex = sp.tile([128, cfg.OUT], F32, tag="ex")
                nc.scalar.activation(ex[:], sh[:], mybir.ActivationFunctionType.Exp)
                sm = mp.tile([128, 1], F32, tag="sm")
                nc.vector.tensor_reduce(sm[:], ex[:], axis=mybir.AxisListType.X,
                                        op=mybir.AluOpType.add)
                nc.scalar.activation(sm[:], sm[:], mybir.ActivationFunctionType.Ln)
                nc.vector.tensor_scalar(sh[:], sh[:], sm[:], None,
                                        op0=mybir.AluOpType.subtract)
                nc.sync.dma_start(out=t_lsm[t * 128:(t + 1) * 128, :], in_=sh[:])

    nc.finalize()
    return nc


# ------------------------------------------------------- cached PJRT runner

def _make_runner(nc, n_cores):
    """Like bass2jax.run_bass_via_pjrt, but builds the jitted executable once
    so repeated calls don't re-trace/re-compile."""
    import jax
    import concourse.mybir as mb
    from jax.sharding import Mesh, PartitionSpec
    from jax.experimental.shard_map import shard_map
    from concourse.bass2jax import (install_neuronx_cc_hook, partition_id_tensor,
                                    _bass_exec_p)
    install_neuronx_cc_hook()
    partition_name = nc.partition_id_tensor.name if nc.partition_id_tensor else None
    in_names, out_names, out_avals, zero_outs = [], [], [], []
    for alloc in nc.m.functions[0].allocations:
        if not isinstance(alloc, mb.MemoryLocationSet):
            continue
        name = alloc.memorylocations[0].name
        if alloc.kind == "ExternalInput":
            if name != partition_name:
                in_names.append(name)
        elif alloc.kind == "ExternalOutput":
            out_names.append(name)
            shape = tuple(alloc.tensor_shape)
            dtype = mb.dt.np(alloc.dtype)
            out_avals.append(jax.core.ShapedArray(shape, dtype))
            zero_outs.append(np.zeros(shape, dtype))
    n_params = len(in_names)
    n_outs = len(out_avals)
    all_in_names = list(in_names) + list(out_names)
    if partition_name is not None:
        all_in_names.append(partition_name)
    donate = tuple(range(n_params, n_params + n_outs))

    def _body(*args):
        operands = list(args)
        if partition_name is not None:
            operands.append(partition_id_tensor())
        return tuple(_bass_exec_p.bind(
            *operands, out_avals=tuple(out_avals), in_names=tuple(all_in_names),
            out_names=tuple(out_names), lowering_input_output_aliases=(),
            sim_require_finite=True, sim_require_nnan=True, nc=nc))

    devices = jax.devices()[:n_cores]
    mesh = Mesh(np.asarray(devices), ("core",))
    in_specs = (PartitionSpec("core"),) * (n_params + n_outs)
    out_specs = (PartitionSpec("core"),) * n_outs
    sharded = jax.jit(
        shard_map(_body, mesh=mesh, in_specs=in_specs, out_specs=out_specs,
                  check_rep=False),
        donate_argnums=donate, keep_unused=True)

    def call(in_maps):
        concat_in = [
            np.concatenate([np.asarray(in_maps[c][k]) for c in range(n_cores)], 0)
            for k in in_names
        ]
        concat_zeros = [
            np.zeros((n_cores * z.shape[0], *z.shape[1:]), z.dtype)
            for z in zero_outs
        ]
        out_arrs = sharded(*concat_in, *concat_zeros)
        jax.block_until_ready(out_arrs)
        return [
            {k: np.asarray(out_arrs[i]).reshape(n_cores, *out_avals[i].shape)[c]
             for i, k in enumerate(out_names)}
            for c in range(n_cores)
        ]

    return call


# Measured by pipelined-dispatch differencing vs an empty-program variant
# (no neuron-profile hooks in this container): full 6.68ms/call - empty
# 5.22ms/call pipelined. Baseline (pre-optimization) was 2127000.
HW_EXEC_NS_ESTIMATE = 1460000

# ----------------------------------------------------------------- entry

_CACHE = {}


def run(cfg, inputs, trace=False):
    ei = np.asarray(inputs["edge_index"])
    key = (cfg.N, cfg.E, cfg.NL, hash(ei.tobytes()))
    if key in _CACHE:
        runner, plan = _CACHE[key]
    else:
        plan = plan_edges(cfg, ei)
        nc = build_program(cfg, plan)
        runner = _make_runner(nc, cfg.NC)
        _CACHE[key] = (runner, plan)
    in_maps = shard_inputs(cfg, inputs, plan)
    results = runner(in_maps)
    lsm = np.concatenate([results[c]["lsm"][:cfg.NV] for c in range(cfg.NC)], 0)
    emb = np.concatenate([results[c]["emb"][:cfg.NV] for c in range(cfg.NC)], 0)
    return (lsm, emb), None


def kernel(**inputs):
    (lsm, emb), _ = run(FULL, inputs)
    return lsm, emb


# revision 5
# speedup vs baseline: 1.7725x; 1.2167x over previous
"""FAGCN (FAConv x3) Trainium2 kernel, 8-core SPMD.

Sharding: nodes partitioned across 8 cores (6250 each, padded to 6272).
Edges assigned to the owner of dst. Per layer each core computes its
slice of the node table (rows = [h*dinv (128 bf16) | al | pad] = 512B),
AllGathers it, then runs an edge pass: dma_gather of table rows by src,
coef = tanh(al_src + ar_dst) * mask, and a one-hot matmul segment-sum on
the TensorEngine (PSUM accumulate per 128-node dst window).
h_new = dinv * segsum + EPS * raw.

Collective/compute overlap: the per-core table is stored as two
row-disjoint tensors (A: rows 0..3200, B: 3200..6272, so each AllGather
output stays under the int16 gather-index reach with no rebasing).
Edges are ordered [locA | locB | remA | remB] (src on own core / remote,
table half A/B), each grouped by 128-node dst window. AllGather-A is
issued as soon as the first 3200 rows are prepped (mid phase-A / mid
combine-loop), AllGather-B at the end; the local segments gather from
the local tables and run while the collectives are in flight, so the
remote segments' gathers rarely wait.

Gather calls are 8 chunks (1024 rows) each -- the SWDGE descriptor
carveout is 16KB = 1024 descriptors, a hard ucode limit -- striped over
4 swdge queues with a GLOBAL call counter (tile_sem_assignment binds
Pool-DMA instructions to DMASW lanes round-robin mod 8 program-wide, so
queue must be counter % 4 to keep each lane on one queue), and 6 gather
buffers to keep several calls in flight.
"""
import numpy as np

import concourse.bacc as bacc
import concourse.bass as bass
import concourse.mybir as mybir
import concourse.tile as tile
from concourse.bass_utils import run_bass_kernel_spmd
from concourse.masks import make_identity

F32 = mybir.dt.float32
BF16 = mybir.dt.bfloat16
I16 = mybir.dt.int16

EPS = 0.1


class Cfg:
    def __init__(self, n_nodes, n_edges, in_dim, out_dim, n_layers,
                 n_cores=8, wa=25, csup=8, nq=4, gbufs=6):
        self.N = n_nodes
        self.E = n_edges
        self.IN = in_dim
        self.H = 128
        self.OUT = out_dim
        self.NL = n_layers
        self.NC = n_cores
        self.NV = n_nodes // n_cores          # owned nodes per core
        assert self.NV * n_cores == n_nodes
        self.W = (self.NV + 127) // 128       # dst windows per core
        self.NP = self.W * 128                # padded nodes per core
        self.KT = in_dim // 128               # k-tiles of the input matmul
        assert in_dim % 128 == 0
        self.WA = wa                          # windows in table half A
        self.RA = wa * 128
        self.RB = self.NP - self.RA
        assert n_cores * self.RA <= 32768 and n_cores * self.RB <= 32768
        self.CSUP = csup                      # chunks per gather call (<=8)
        self.NQ = nq                          # swdge queues (<=4)
        self.GBUFS = gbufs                    # gather tile buffers


FULL = Cfg(50000, 600000, 512, 64, 3)


# ----------------------------------------------------------------- planner

def plan_edges(cfg, edge_index):
    """Host-side edge sharding: 4 segments x dst-window groups, uniform
    (max-over-cores) chunk schedule so the SPMD program is shared."""
    src = edge_index[0].astype(np.int64)
    dst = edge_index[1].astype(np.int64)
    owner = dst // cfg.NV

    NSEG = 4
    per_core = []
    counts = np.zeros((cfg.NC, NSEG, cfg.W), np.int64)
    for c in range(cfg.NC):
        m = owner == c
        s = src[m]
        d_l = dst[m] - c * cfg.NV
        w = d_l >> 7
        c_src = s // cfg.NV
        r = s % cfg.NV
        seg = np.where(c_src == c,
                       np.where(r < cfg.RA, 0, 1),
                       np.where(r < cfg.RA, 2, 3))
        order = np.lexsort((d_l, w, seg))
        s, d_l, w, seg, c_src, r = (a[order] for a in (s, d_l, w, seg, c_src, r))
        gv = np.select(
            [seg == 0, seg == 1, seg == 2, seg == 3],
            [r, r - cfg.RA, c_src * cfg.RA + r, c_src * cfg.RB + (r - cfg.RA)])
        for sg in range(NSEG):
            for ww in range(cfg.W):
                counts[c, sg, ww] = np.count_nonzero((seg == sg) & (w == ww))
        per_core.append((gv, d_l))

    nch = np.maximum((counts.max(axis=0) + 127) // 128, 0)
    nch[counts.max(axis=0) == 0] = 0
    NCH = int(nch.sum())
    EPAD = NCH * 128

    chunk_meta = []       # (seg, window, first_of_group, last_of_group)
    first_grp = {}        # window -> first segment with chunks
    for sg in range(NSEG):
        for ww in range(cfg.W):
            n = int(nch[sg, ww])
            if n and ww not in first_grp:
                first_grp[ww] = sg
            for k in range(n):
                chunk_meta.append((sg, ww, k == 0, k == n - 1))
    seg_end = np.cumsum(nch.sum(axis=1)).astype(int)

    cores = []
    for c in range(cfg.NC):
        gv, d_l = per_core[c]
        gidx = np.zeros(EPAD, np.int64)
        rel = np.full(EPAD, 999.0, np.float32)  # dst rel in window; 999 = pad
        pos = 0
        ptr = 0
        for sg in range(NSEG):
            for ww in range(cfg.W):
                n = counts[c, sg, ww]
                sl = slice(ptr, ptr + n)
                gidx[pos:pos + n] = gv[sl]
                rel[pos:pos + n] = (d_l[sl] & 127).astype(np.float32)
                ptr += n
                pos += int(nch[sg, ww]) * 128
        assert ptr == len(gv)

        def wrap16(v):
            a = v.astype(np.int16).reshape(-1, 16).T.copy()
            return np.tile(a, (8, 1))

        def lanes(v):
            return v.reshape(-1, 128).T.copy()

        cores.append(dict(gidx=wrap16(gidx), rel=lanes(rel)))
    return dict(nch=nch, NCH=NCH, EPAD=EPAD, seg_end=list(seg_end),
                chunk_meta=chunk_meta, first_grp=first_grp, cores=cores)


def shard_inputs(cfg, inputs, plan):
    """Build per-core in_maps from full inputs."""
    x = np.asarray(inputs["x"], np.float32)
    ei = np.asarray(inputs["edge_index"])
    t1_w = np.asarray(inputs["t1_w"], np.float32)
    t1_b = np.asarray(inputs["t1_b"], np.float32)
    t2_w = np.asarray(inputs["t2_w"], np.float32)
    t2_b = np.asarray(inputs["t2_b"], np.float32)
    att_l = np.asarray(inputs["att_l"], np.float32)
    att_r = np.asarray(inputs["att_r"], np.float32)

    deg_all = np.bincount(ei[1].astype(np.int64), minlength=cfg.N).astype(np.float32)

    w1t = t1_w.T.copy()
    w1t_tiles = w1t.reshape(cfg.KT, 128, cfg.H)
    b1rep = np.broadcast_to(t1_b, (128, cfg.H)).copy()
    alrep = np.stack([np.broadcast_to(att_l[i % att_l.shape[0]], (128, cfg.H))
                      for i in range(cfg.NL)])
    arrep = np.stack([np.broadcast_to(att_r[i % att_r.shape[0]], (128, cfg.H))
                      for i in range(cfg.NL)])
    t2wt = t2_w.T.copy()
    b2rep = np.broadcast_to(t2_b, (128, cfg.OUT)).copy()
    iota = np.broadcast_to(np.arange(128, dtype=np.float32), (128, 128)).copy()

    in_maps = []
    for c in range(cfg.NC):
        lo = c * cfg.NV
        xc = np.zeros((cfg.NP, cfg.IN), np.float32)
        xc[:cfg.NV] = x[lo:lo + cfg.NV]
        xt = xc.reshape(cfg.W, 128, cfg.KT, 128).transpose(0, 3, 2, 1).copy()
        deg = np.zeros(cfg.NP, np.float32)
        deg[:cfg.NV] = deg_all[lo:lo + cfg.NV]
        pc = plan["cores"][c]
        in_maps.append(dict(
            xt=xt, deg=deg,
            w1t=w1t_tiles, b1rep=b1rep, alrep=alrep, arrep=arrep,
            t2wt=t2wt, b2rep=b2rep, iota=iota,
            gidx=pc["gidx"], rel=pc["rel"],
        ))
    return in_maps


# ----------------------------------------------------------------- builder

def build_program(cfg, plan, skip=frozenset()):
    NCH = plan["NCH"]
    meta = plan["chunk_meta"]
    first_grp = plan["first_grp"]
    seg_end = plan["seg_end"]
    EPAD = plan["EPAD"]
    W = cfg.W
    WA = cfg.WA
    WB = W - WA
    RWE = 256  # bf16 row: 512B = [hs(128) | al | pad]

    nc = bacc.Bacc("TRN2", target_bir_lowering=False, debug=False,
                   num_devices=cfg.NC, num_swdge_queues=cfg.NQ)

    t_xt = nc.dram_tensor("xt", [W, 128, cfg.KT, 128], F32, kind="ExternalInput")
    t_deg = nc.dram_tensor("deg", [cfg.NP], F32, kind="ExternalInput")
    t_w1t = nc.dram_tensor("w1t", [cfg.KT, 128, cfg.H], F32, kind="ExternalInput")
    t_b1 = nc.dram_tensor("b1rep", [128, cfg.H], F32, kind="ExternalInput")
    t_al = nc.dram_tensor("alrep", [cfg.NL, 128, cfg.H], F32, kind="ExternalInput")
    t_ar = nc.dram_tensor("arrep", [cfg.NL, 128, cfg.H], F32, kind="ExternalInput")
    t_t2 = nc.dram_tensor("t2wt", [cfg.H, cfg.OUT], F32, kind="ExternalInput")
    t_b2 = nc.dram_tensor("b2rep", [128, cfg.OUT], F32, kind="ExternalInput")
    t_iota = nc.dram_tensor("iota", [128, 128], F32, kind="ExternalInput")
    t_gidx = nc.dram_tensor("gidx", [128, EPAD // 16], I16, kind="ExternalInput")
    t_rel = nc.dram_tensor("rel", [128, NCH], F32, kind="ExternalInput")
    t_lsm = nc.dram_tensor("lsm", [cfg.NP, cfg.OUT], F32, kind="ExternalOutput")
    t_emb = nc.dram_tensor("emb", [cfg.NP, cfg.OUT], F32, kind="ExternalOutput")

    d_locA = nc.dram_tensor("tab_locA", [cfg.RA, RWE], BF16)
    d_locB = nc.dram_tensor("tab_locB", [cfg.RB, RWE], BF16)
    d_fullA = nc.dram_tensor("tab_fullA", [cfg.NC * cfg.RA, RWE], BF16,
                             addr_space="Shared")
    d_fullB = nc.dram_tensor("tab_fullB", [cfg.NC * cfg.RB, RWE], BF16,
                             addr_space="Shared")
    d_ar_loc = nc.dram_tensor("ar_loc", [cfg.NP], F32)

    CS = cfg.CSUP
    rg = [list(range(cfg.NC))]

    with tile.TileContext(nc) as tc:
        with (
            tc.tile_pool(name="const", bufs=1) as cp,
            tc.tile_pool(name="stage", bufs=4) as sp,
            tc.tile_pool(name="xld", bufs=2) as xp,
            tc.tile_pool(name="gath", bufs=cfg.GBUFS) as gp,
            tc.tile_pool(name="oh", bufs=8) as op,
            tc.tile_pool(name="small", bufs=4) as mp,
            tc.tile_pool(name="psum", bufs=6, space="PSUM") as pp,
        ):
            w1 = cp.tile([128, cfg.KT, cfg.H], F32, tag="w1")
            nc.sync.dma_start(out=w1[:], in_=t_w1t[:].rearrange("k p h -> p k h"))
            b1 = cp.tile([128, cfg.H], F32, tag="b1")
            nc.sync.dma_start(out=b1[:], in_=t_b1[:])
            alr = cp.tile([128, cfg.NL, cfg.H], F32, tag="alr")
            nc.sync.dma_start(out=alr[:], in_=t_al[:].rearrange("l p h -> p l h"))
            arr = cp.tile([128, cfg.NL, cfg.H], F32, tag="arr")
            nc.sync.dma_start(out=arr[:], in_=t_ar[:].rearrange("l p h -> p l h"))
            t2w = cp.tile([cfg.H, cfg.OUT], F32, tag="t2w")
            nc.sync.dma_start(out=t2w[:], in_=t_t2[:])
            b2 = cp.tile([128, cfg.OUT], F32, tag="b2")
            nc.sync.dma_start(out=b2[:], in_=t_b2[:])
            iota = cp.tile([128, 128], F32, tag="iota")
            nc.sync.dma_start(out=iota[:], in_=t_iota[:])
            gidx = cp.tile([128, EPAD // 16], I16, tag="gidx")
            nc.sync.dma_start(out=gidx[:], in_=t_gidx[:])
            ones1 = cp.tile([1, 128], F32, tag="ones1")
            nc.vector.memset(ones1[:], 1.0)
            rel = cp.tile([128, NCH], F32, tag="rel")
            nc.sync.dma_start(out=rel[:], in_=t_rel[:])
            ident = cp.tile([128, 128], F32, tag="ident")
            make_identity(nc, ident[:])

            h_sb = cp.tile([128, W, cfg.H], F32, tag="h")
            raw_sb = cp.tile([128, W, cfg.H], F32, tag="raw")
            acc_sb = cp.tile([128, W, cfg.H], F32, tag="acc")
            dinv = cp.tile([128, W], F32, tag="dinv")
            alc = cp.tile([128, W], F32, tag="alc")
            arc = cp.tile([128, W], F32, tag="arc")

            if "gather0" in skip:
                ghs0 = cp.tile([128, CS * RWE], BF16, tag="ghs0")
                nc.vector.memset(ghs0[:], 0.0)

            # ---------- dinv = (deg>0) / sqrt(max(deg,1))
            degt = mp.tile([128, W], F32, tag="degt")
            with nc.allow_non_contiguous_dma(reason="node-col load"):
                nc.sync.dma_start(out=degt[:], in_=t_deg[:].rearrange("(t p) -> p t", p=128))
            dmax = mp.tile([128, W], F32, tag="dmax")
            nc.vector.tensor_scalar_max(dmax[:], degt[:], 1.0)
            nc.scalar.sqrt(dmax[:], dmax[:])
            nc.vector.reciprocal(dmax[:], dmax[:])
            dnz = mp.tile([128, W], F32, tag="dnz")
            nc.vector.tensor_scalar(dnz[:], degt[:], 0.0, None,
                                    op0=mybir.AluOpType.is_gt)
            nc.vector.tensor_tensor(out=dinv[:], in0=dmax[:], in1=dnz[:],
                                    op=mybir.AluOpType.mult)

            def nprep(t, li):
                """al/ar accum + hs row store for window t of layer li."""
                tmp = sp.tile([128, cfg.H], F32, tag="nprep")
                nc.vector.scalar_tensor_tensor(
                    tmp[:], h_sb[:, t, :], 1.0, alr[:, li, :],
                    op0=mybir.AluOpType.mult, op1=mybir.AluOpType.mult,
                    accum_out=alc[:, t:t + 1])
                nc.vector.scalar_tensor_tensor(
                    tmp[:], h_sb[:, t, :], 1.0, arr[:, li, :],
                    op0=mybir.AluOpType.mult, op1=mybir.AluOpType.mult,
                    accum_out=arc[:, t:t + 1])
                hst = sp.tile([128, cfg.H], BF16, tag="hst")
                nc.vector.tensor_scalar_mul(hst[:], h_sb[:, t, :], dinv[:, t:t + 1])
                if t < WA:
                    dst_ap = d_locA[t * 128:(t + 1) * 128, :cfg.H]
                else:
                    tb = t - WA
                    dst_ap = d_locB[tb * 128:(tb + 1) * 128, :cfg.H]
                nc.sync.dma_start(out=dst_ap, in_=hst[:])

            def colstore_a():
                alx = sp.tile([128, WA], BF16, tag="alxA")
                nc.vector.tensor_copy(alx[:], alc[:, :WA])
                with nc.allow_non_contiguous_dma(reason="al-col store A"):
                    nc.sync.dma_start(
                        out=d_locA[:, cfg.H:cfg.H + 1].rearrange(
                            "(t p) c -> p (t c)", p=128),
                        in_=alx[:])

            def colstore_b():
                alx = sp.tile([128, WB], BF16, tag="alxB")
                nc.vector.tensor_copy(alx[:], alc[:, WA:])
                with nc.allow_non_contiguous_dma(reason="al-col store B"):
                    nc.sync.dma_start(
                        out=d_locB[:, cfg.H:cfg.H + 1].rearrange(
                            "(t p) c -> p (t c)", p=128),
                        in_=alx[:])

            def ag_a():
                if "ag" not in skip:
                    nc.gpsimd.collective_compute(
                        "AllGather", mybir.AluOpType.bypass, replica_groups=rg,
                        ins=[d_locA[:]], outs=[d_fullA[:]])

            def ag_b():
                if "ag" not in skip:
                    nc.gpsimd.collective_compute(
                        "AllGather", mybir.AluOpType.bypass, replica_groups=rg,
                        ins=[d_locB[:]], outs=[d_fullB[:]])

            # ---------- phase A: h = relu(x@W1+b1), fused layer-0 nprep + AGs
            AB = 7
            for t0 in range(0, W if "phasea" not in skip else 0, AB):
                nb = min(AB, W - t0)
                xa = xp.tile([128, AB * cfg.KT * 128], F32, tag="xa")
                nc.sync.dma_start(
                    out=xa[:, :nb * cfg.KT * 128],
                    in_=t_xt[t0:t0 + nb].rearrange("w p k n -> p w k n"))
                for ti in range(nb):
                    t = t0 + ti
                    ps = pp.tile([128, cfg.H], F32, tag="ps")
                    for k in range(cfg.KT):
                        o = (ti * cfg.KT + k) * 128
                        nc.tensor.matmul(ps[:], lhsT=xa[:, o:o + 128],
                                         rhs=w1[:, k, :],
                                         start=(k == 0), stop=(k == cfg.KT - 1))
                    hb = sp.tile([128, cfg.H], F32, tag="hb")
                    nc.vector.tensor_add(hb[:], ps[:], b1[:])
                    nc.scalar.activation(h_sb[:, t, :], hb[:],
                                         mybir.ActivationFunctionType.Relu)
                    nc.scalar.mul(raw_sb[:, t, :], h_sb[:, t, :], EPS)
                    if "nprep" not in skip:
                        nprep(t, 0)
                        if t == WA - 1:
                            colstore_a()
                            ag_a()
                        if t == W - 1:
                            colstore_b()
                            ag_b()

            # ---------- layers
            # Global gather-call counter: tile_sem_assignment binds Pool-DMA
            # instructions to DMASW lanes round-robin mod 8 program-wide, so
            # queue must be (global counter) % NQ to keep each lane on a
            # single queue (NQ divides 8).
            gcall = [0]

            def gather_tab(sg):
                return [d_locA[:], d_locB[:], d_fullA[:], d_fullB[:]][sg]

            for li in range(cfg.NL):
                # ar flatten roundtrip: arc [128, W] -> d_ar_loc -> arct [1, NP]
                arct = cp.tile([1, cfg.NP], F32, tag="arct")
                if "nprep" not in skip:
                    with nc.allow_non_contiguous_dma(reason="ar-col store"):
                        nc.sync.dma_start(
                            out=d_ar_loc[:].rearrange("(t p) -> p t", p=128),
                            in_=arc[:])
                    nc.sync.dma_start(out=arct[:], in_=d_ar_loc[None, :])
                elif "chunk" not in skip:
                    nc.vector.memset(arct[:], 0.0)

                # edge pass
                psw = None
                psar = None
                cur_group = None
                c0 = 0
                while c0 < NCH:
                    bound = next(b for b in seg_end if b > c0)
                    nch_call = min(CS, NCH - c0, bound - c0)
                    sg = meta[c0][0]
                    ne = nch_call * 128
                    if "gather0" in skip:
                        ghs = ghs0
                    else:
                        ghs = gp.tile([128, CS * RWE], BF16, tag="ghs")
                        i0, i1 = c0 * 8, (c0 + nch_call) * 8
                        nc.gpsimd.dma_gather(
                            out_ap=ghs[:, :nch_call * RWE].rearrange(
                                "p (c e) -> p c e", e=RWE),
                            in_ap=gather_tab(sg), idxs_ap=gidx[:, i0:i1],
                            num_idxs=ne, num_idxs_reg=ne, elem_size=RWE,
                            queue_num=gcall[0] % cfg.NQ)
                        gcall[0] += 1
                    for j in range(nch_call if "chunk" not in skip else 0):
                        ci = c0 + j
                        sgj, ww, first, last = meta[ci]
                        if (sgj, ww) != cur_group:
                            # ar_rep[p, n] = ar[window, node n] via rank-1 mm
                            psar = pp.tile([128, 128], F32, tag="ps")
                            nc.tensor.matmul(
                                psar[:], lhsT=ones1[:],
                                rhs=arct[0:1, ww * 128:(ww + 1) * 128],
                                start=True, stop=True)
                            cur_group = (sgj, ww)
                        tt = op.tile([128, 128], F32, tag="tt")
                        nc.scalar.activation(
                            tt[:], psar[:], mybir.ActivationFunctionType.Tanh,
                            bias=ghs[:, j * RWE + cfg.H:j * RWE + cfg.H + 1])
                        ohp = op.tile([128, 128], BF16, tag="ohp")
                        nc.vector.scalar_tensor_tensor(
                            ohp[:], iota[:], rel[:, ci:ci + 1], tt[:],
                            op0=mybir.AluOpType.is_equal,
                            op1=mybir.AluOpType.mult)
                        if first:
                            psw = pp.tile([128, cfg.H], F32, tag="ps")
                        nc.tensor.matmul(psw[:], lhsT=ohp[:],
                                         rhs=ghs[:, j * RWE:j * RWE + cfg.H],
                                         start=first, stop=last)
                        if last:
                            if first_grp.get(ww) == sgj:
                                nc.vector.tensor_copy(acc_sb[:, ww, :], psw[:])
                            else:
                                nc.vector.tensor_add(acc_sb[:, ww, :],
                                                     acc_sb[:, ww, :], psw[:])
                    c0 += nch_call

                # combine + next-layer nprep fused; early AGs for layer li+1
                for t in range(W if "nprep" not in skip else 0):
                    if t in first_grp:
                        nc.vector.scalar_tensor_tensor(
                            h_sb[:, t, :], acc_sb[:, t, :], dinv[:, t:t + 1],
                            raw_sb[:, t, :],
                            op0=mybir.AluOpType.mult, op1=mybir.AluOpType.add)
                    else:
                        nc.vector.tensor_copy(h_sb[:, t, :], raw_sb[:, t, :])
                    if li + 1 < cfg.NL:
                        nprep(t, li + 1)
                        if t == WA - 1:
                            colstore_a()
                            ag_a()
                        if t == W - 1:
                            colstore_b()
                            ag_b()

            # ---------- phase C: emb = h @ t2_w.T + b2; lsm = log_softmax
            for t in range(W if "phasec" not in skip else 0):
                pst = pp.tile([128, 128], F32, tag="ps")
                nc.tensor.transpose(out=pst[:], in_=h_sb[:, t, :], identity=ident[:])
                ht = sp.tile([128, 128], F32, tag="ht")
                nc.vector.tensor_copy(ht[:], pst[:])
                pse = pp.tile([128, cfg.OUT], F32, tag="ps")
                nc.tensor.matmul(pse[:], lhsT=ht[:], rhs=t2w[:], start=True, stop=True)
                emb = sp.tile([128, cfg.OUT], F32, tag="embt")
                nc.vector.tensor_add(emb[:], pse[:], b2[:])
                nc.sync.dma_start(out=t_emb[t * 128:(t + 1) * 128, :], in_=emb[:])
                mx = mp.tile([128, 1], F32, tag="mx")
                nc.vector.tensor_reduce(mx[:], emb[:], axis=mybir.AxisListType.X,
                                        op=mybir.AluOpType.max)
                sh = sp.tile([128, cfg.OUT], F32, tag="sh")
                nc.vector.tensor_scalar(sh[:], emb[:], mx[:], None,
                                        op0=mybir.AluOpType.subtract)
                ex = sp.tile([128, cfg.OUT], F32, tag="ex")
                nc.scalar.activation(ex[:], sh[:], mybir.ActivationFunctionType.Exp)
                sm = mp.tile([128, 1], F32, tag="sm")
                nc.vector.tensor_reduce(sm[:], ex[:], axis=mybir.AxisListType.X,
                                        op=mybir.AluOpType.add)
                nc.scalar.activation(sm[:], sm[:], mybir.ActivationFunctionType.Ln)
                nc.vector.tensor_scalar(sh[:], sh[:], sm[:], None,
                                        op0=mybir.AluOpType.subtract)
                nc.sync.dma_start(out=t_lsm[t * 128:(t + 1) * 128, :], in_=sh[:])

    nc.finalize()
    return nc


# ------------------------------------------------------- cached PJRT runner

def _make_runner(nc, n_cores):
    """Like bass2jax.run_bass_via_pjrt, but builds the jitted executable once
    so repeated calls don't re-trace/re-compile."""
    import jax
    import concourse.mybir as mb
    from jax.sharding import Mesh, PartitionSpec
    from jax.experimental.shard_map import shard_map
    from concourse.bass2jax import (install_neuronx_cc_hook, partition_id_tensor,
                                    _bass_exec_p)
    install_neuronx_cc_hook()
    partition_name = nc.partition_id_tensor.name if nc.partition_id_tensor else None
    in_names, out_names, out_avals, zero_outs = [], [], [], []
    for alloc in nc.m.functions[0].allocations:
        if not isinstance(alloc, mb.MemoryLocationSet):
            continue
        name = alloc.memorylocations[0].name
        if alloc.kind == "ExternalInput":
            if name != partition_name:
                in_names.append(name)
        elif alloc.kind == "ExternalOutput":
            out_names.append(name)
            shape = tuple(alloc.tensor_shape)
            dtype = mb.dt.np(alloc.dtype)
            out_avals.append(jax.core.ShapedArray(shape, dtype))
            zero_outs.append(np.zeros(shape, dtype))
    n_params = len(in_names)
    n_outs = len(out_avals)
    all_in_names = list(in_names) + list(out_names)
    if partition_name is not None:
        all_in_names.append(partition_name)
    donate = tuple(range(n_params, n_params + n_outs))

    def _body(*args):
        operands = list(args)
        if partition_name is not None:
            operands.append(partition_id_tensor())
        return tuple(_bass_exec_p.bind(
            *operands, out_avals=tuple(out_avals), in_names=tuple(all_in_names),
            out_names=tuple(out_names), lowering_input_output_aliases=(),
            sim_require_finite=True, sim_require_nnan=True, nc=nc))

    devices = jax.devices()[:n_cores]
    mesh = Mesh(np.asarray(devices), ("core",))
    in_specs = (PartitionSpec("core"),) * (n_params + n_outs)
    out_specs = (PartitionSpec("core"),) * n_outs
    sharded = jax.jit(
        shard_map(_body, mesh=mesh, in_specs=in_specs, out_specs=out_specs,
                  check_rep=False),
        donate_argnums=donate, keep_unused=True)

    def call(in_maps):
        concat_in = [
            np.concatenate([np.asarray(in_maps[c][k]) for c in range(n_cores)], 0)
            for k in in_names
        ]
        concat_zeros = [
            np.zeros((n_cores * z.shape[0], *z.shape[1:]), z.dtype)
            for z in zero_outs
        ]
        out_arrs = sharded(*concat_in, *concat_zeros)
        jax.block_until_ready(out_arrs)
        return [
            {k: np.asarray(out_arrs[i]).reshape(n_cores, *out_avals[i].shape)[c]
             for i, k in enumerate(out_names)}
            for c in range(n_cores)
        ]

    return call


# Measured by pipelined-dispatch differencing (no neuron-profile hooks in
# this container). The per-session dispatch constant drifts ~1ms, so the
# estimate anchors on same-session deltas: this kernel runs 0.9-1.0ms/call
# faster than the 2127000ns baseline; a same-session full-vs-empty delta
# gives 690000ns (lower bound; dispatch may rate-limit the empty program).
HW_EXEC_NS_ESTIMATE = 1200000

# ----------------------------------------------------------------- entry

_CACHE = {}


def run(cfg, inputs, trace=False):
    ei = np.asarray(inputs["edge_index"])
    key = (cfg.N, cfg.E, cfg.NL, hash(ei.tobytes()))
    if key in _CACHE:
        runner, plan = _CACHE[key]
    else:
        plan = plan_edges(cfg, ei)
        nc = build_program(cfg, plan)
        runner = _make_runner(nc, cfg.NC)
        _CACHE[key] = (runner, plan)
    in_maps = shard_inputs(cfg, inputs, plan)
    results = runner(in_maps)
    lsm = np.concatenate([results[c]["lsm"][:cfg.NV] for c in range(cfg.NC)], 0)
    emb = np.concatenate([results[c]["emb"][:cfg.NV] for c in range(cfg.NC)], 0)
    return (lsm, emb), None


def kernel(**inputs):
    (lsm, emb), _ = run(FULL, inputs)
    return lsm, emb
